# revision 13
# baseline (speedup 1.0000x reference)
"""Trainium2 Bass kernel for nn_Linear_18494129177115 (moe_routing).

Math (reference, fp32):
  base   = x @ W^T                                  [B,T,O]
  logits = x @ Wr^T + lang_bias                     [B,T,E]
  gates  = scatter(softmax(top2(logits)))           [B,T,E]
  h      = x @ A_cat^T  (all experts)               [B,T,E*R]
  out    = base + (gates_expanded * h) @ (SCALING * B_cat)

Key design points vs the previous 3-pass version:
- out^T formulation: out^T[o,t] tiles accumulate in PSUM with the W^T
  block [128d x 128o] as the PE stationary operand and x^T as the moving
  operand. Each stationary load covers 1024 moving columns (all tokens),
  so LDWEIGHTS cost is ~10% and hidden by the PE reorder window.
- Single-pass bf16 for the base GEMM (error ~2e-3 absmax-rel, measured
  1.7e-3 end-to-end vs the 2e-2 gate): bf16 products are exact in fp32,
  only input rounding contributes. This is 3x fewer PE cycles than the
  hi/lo 3-pass split and needs only W_hi from HBM (32 MB vs 64 MB).
- Router logits in 3-pass bf16 (xh@wrh + xl@wrh + xh@wrl, err ~4e-6) so
  top-2 selection is faithful: 1-pass flips rare selections and costs
  ~1e-2 absmax; 3-pass measured zero flips on 8192 tokens.
- LoRA h / B_cat matmuls single-pass bf16, fused into the same PSUM
  accumulation group as the base GEMM (B_cat pre-scaled by SCALING).
- W streamed per 128-col block as one contiguous 1 MB DMA (8 KB per
  partition line), triple-buffered, alternating the two HWDGE rings.

Sharding: data-parallel over tokens, 1024 tokens/core on 8 cores; all
weights replicated; no collectives. Each core's tokens lie in a single
batch row, so the language bias is a per-core constant column [E,1].
"""

import numpy as np

LANG_BIAS = 5.0
SCALING = 32.0 / 16.0
B_SZ, T_SZ, D_SZ, O_SZ, E_SZ, R_SZ = 4, 2048, 4096, 4096, 8, 16
NCORES = 8
TPC = (B_SZ * T_SZ) // NCORES      # 1024 tokens per core
NT = TPC // 128                    # 8 token tiles per core
NK = D_SZ // 128                   # 32 contraction chunks
NO2 = O_SZ // 128                  # 32 output tiles of 128
ER = E_SZ * R_SZ                   # 128 (expert, rank) pairs

_CACHE: dict = {}
LAST_RESULT = None


def _build_bass(loop_n: int | None = None, parts=("p1", "p2split")):
    import concourse.bacc as bacc
    import concourse.mybir as mybir
    from concourse import tile
    from concourse.masks import make_identity

    f32 = mybir.dt.float32
    bf16 = mybir.dt.bfloat16
    AX = mybir.AxisListType.X
    OP = mybir.AluOpType
    ACT = mybir.ActivationFunctionType

    nc = bacc.Bacc(None, target_bir_lowering=False, debug=False)

    # x^T hi/lo: [2(hi/lo), kc, p(d), t]
    xt_d = nc.dram_tensor("xt", [2, NK, 128, TPC], bf16, kind="ExternalInput")
    # W^T blocks: [ot, p(d), kc, o] -- 8KB contiguous per partition line
    wt_d = nc.dram_tensor("wt", [NO2, 128, NK, 128], bf16, kind="ExternalInput")
    # Wr^T hi/lo: [2, p(d), kc, e]
    wrt_d = nc.dram_tensor("wrt", [2, 128, NK, E_SZ], bf16, kind="ExternalInput")
    # A_cat^T: [p(d), kc, er]
    acat_d = nc.dram_tensor("acat", [128, NK, ER], bf16, kind="ExternalInput")
    # SCALING * B_cat: [er, o]
    bcat_d = nc.dram_tensor("bcat", [ER, O_SZ], bf16, kind="ExternalInput")
    selw_d = nc.dram_tensor("selw", [E_SZ, ER], f32, kind="ExternalInput")
    biasc_d = nc.dram_tensor("biasc", [E_SZ, 1], f32, kind="ExternalInput")
    # out^T: [ot, p(o), t]
    out_d = nc.dram_tensor("out", [NO2, 128, TPC], f32, kind="ExternalOutput")

    with tile.TileContext(nc) as tc:
        with (
            tc.tile_pool(name="const", bufs=1) as cpool,
            tc.tile_pool(name="wstream", bufs=3) as wpool,
            tc.tile_pool(name="ostage", bufs=2) as opool,
            tc.tile_pool(name="gate", bufs=2) as gpool,
            tc.tile_pool(name="psum", bufs=8, space="PSUM") as psum,
        ):

          def body(_iv=None):
            # ---- resident inputs ----
            xh_sb = cpool.tile([128, NK, TPC], bf16, name="xh_sb")
            xl_sb = cpool.tile([128, NK, TPC], bf16, name="xl_sb")
            wrh_sb = cpool.tile([128, NK, E_SZ], bf16, name="wrh_sb")
            wrl_sb = cpool.tile([128, NK, E_SZ], bf16, name="wrl_sb")
            acat_sb = cpool.tile([128, NK, ER], bf16, name="acat_sb")
            bcat_sb = cpool.tile([ER, O_SZ], bf16, name="bcat_sb")
            sel_sb = cpool.tile([E_SZ, ER], f32, name="sel_sb")
            biasc_sb = cpool.tile([E_SZ, 1], f32, name="biasc_sb")
            ident_sb = cpool.tile([128, 128], f32, name="ident_sb")
            ident8_sb = cpool.tile([8, 8], f32, name="ident8_sb")
            hT_sb = cpool.tile([128, TPC], f32, name="hT_sb")
            ghT_sb = cpool.tile([128, TPC], bf16, name="ghT_sb")
            lgb_sb = cpool.tile([E_SZ, TPC], f32, name="lgb_sb")
            gT_sb = cpool.tile([E_SZ, TPC], f32, name="gT_sb")

            do_p1 = "p1" in parts
            do_rt = "p2rt" in parts
            do_p2 = "p2" in parts or "p2split" in parts or do_rt
            do_xdma = "noxdma" not in parts

            # x hi on the two HWDGE rings, x lo (router-only) on SWDGE
            for g in range(4 if do_xdma else 0):
                ksl = slice(g * 8, (g + 1) * 8)
                eng = nc.sync if g % 2 == 0 else nc.scalar
                eng.dma_start(
                    xh_sb[:, ksl, :], xt_d[0, ksl].rearrange("k p t -> p k t")
                )
                if do_p1:
                    nc.gpsimd.dma_start(
                        xl_sb[:, ksl, :], xt_d[1, ksl].rearrange("k p t -> p k t")
                    )
            nc.scalar.dma_start(wrh_sb[:], wrt_d[0])
            nc.scalar.dma_start(wrl_sb[:], wrt_d[1])
            nc.scalar.dma_start(acat_sb[:], acat_d[:])
            nc.sync.dma_start(bcat_sb[:], bcat_d[:])
            nc.scalar.dma_start(sel_sb[:], selw_d[:])
            nc.scalar.dma_start(biasc_sb[:], biasc_d[:])
            make_identity(nc, ident_sb[:])
            make_identity(nc, ident8_sb[:])

            if not do_p1:
                nc.any.memset(ghT_sb[:], 0.0)

            # ---- phase 1a: router logits^T [E, TPC], 3-pass bf16 ----
            plT = [
                psum.tile([E_SZ, 512], f32, tag="bank", name=f"plT{t}")
                for t in range(2)
            ]
            for kc in range(NK if do_p1 else 0):
                wh = wrh_sb[:, kc, :]
                wl = wrl_sb[:, kc, :]
                first = kc == 0
                last = kc == NK - 1
                for tb in range(2):
                    sl = slice(tb * 512, (tb + 1) * 512)
                    nc.tensor.matmul(
                        plT[tb][:], wh, xh_sb[:, kc, sl], start=first, stop=False
                    )
                    nc.tensor.matmul(
                        plT[tb][:], wh, xl_sb[:, kc, sl], start=False, stop=False
                    )
                for tb in range(2):
                    sl = slice(tb * 512, (tb + 1) * 512)
                    nc.tensor.matmul(
                        plT[tb][:], wl, xh_sb[:, kc, sl], start=False, stop=last
                    )
            # bias add during PSUM->SBUF move (per-partition scalar on 8 rows)
            for tb in range(2 if do_p1 else 0):
                sl = slice(tb * 512, (tb + 1) * 512)
                nc.vector.tensor_scalar(
                    lgb_sb[:, sl], plT[tb][:], biasc_sb[:], None, op0=OP.add
                )

            # ---- phase 1b: h^T = A_cat @ x^T, 1-pass bf16 ----
            ph = [
                psum.tile([128, 512], f32, tag="bank", name=f"ph{t}")
                for t in range(2)
            ]
            for kc in range(NK if do_p1 else 0):
                a = acat_sb[:, kc, :]
                first = kc == 0
                last = kc == NK - 1
                for tb in range(2):
                    sl = slice(tb * 512, (tb + 1) * 512)
                    nc.tensor.matmul(
                        ph[tb][:], a, xh_sb[:, kc, sl], start=first, stop=last
                    )
            for tb in range(2 if do_p1 else 0):
                sl = slice(tb * 512, (tb + 1) * 512)
                nc.vector.tensor_copy(hT_sb[:, sl], ph[tb][:])

            # ---- phase 1c: per-tile top-2 softmax gates ----
            ptr_pack = [
                psum.tile([E_SZ, 512], f32, tag="bank", name=f"ptr{t}")
                for t in range(2)
            ]
            for tt in range(NT if do_p1 else 0):
                ts = slice(tt * 128, (tt + 1) * 128)
                plg = psum.tile([128, E_SZ], f32, tag="bank", name=f"plg{tt}")
                nc.tensor.transpose(plg[:], lgb_sb[:, ts], ident8_sb[:])
                logit = gpool.tile([128, E_SZ], f32, name="logit")
                nc.vector.tensor_copy(logit[:], plg[:])
                m1 = gpool.tile([128, 1], f32, name="m1")
                nc.vector.reduce_max(m1[:], logit[:], axis=AX)
                mask1 = gpool.tile([128, E_SZ], f32, name="mask1")
                nc.vector.tensor_scalar(
                    mask1[:], logit[:], m1[:], None, op0=OP.is_equal
                )
                l2 = gpool.tile([128, E_SZ], f32, name="l2")
                nc.vector.scalar_tensor_tensor(
                    l2[:], mask1[:], -1e30, logit[:], op0=OP.mult, op1=OP.add
                )
                m2 = gpool.tile([128, 1], f32, name="m2")
                nc.vector.reduce_max(m2[:], l2[:], axis=AX)
                mask2 = gpool.tile([128, E_SZ], f32, name="mask2")
                nc.vector.tensor_scalar(
                    mask2[:], l2[:], m2[:], None, op0=OP.is_equal
                )
                w1 = gpool.tile([128, 1], f32, name="w1")
                nc.scalar.activation(
                    w1[:], m2[:], ACT.Sigmoid, bias=m1[:], scale=-1.0
                )
                w2 = gpool.tile([128, 1], f32, name="w2")
                nc.vector.tensor_scalar(
                    w2[:], w1[:], -1.0, 1.0, op0=OP.mult, op1=OP.add
                )
                g1 = gpool.tile([128, E_SZ], f32, name="g1")
                nc.vector.tensor_scalar(g1[:], mask1[:], w1[:], None, op0=OP.mult)
                gates = gpool.tile([128, E_SZ], f32, name="gates")
                nc.vector.scalar_tensor_tensor(
                    gates[:], mask2[:], w2[:], g1[:], op0=OP.mult, op1=OP.add
                )
                nc.tensor.transpose(
                    ptr_pack[tt // 4][:, (tt % 4) * 128 : (tt % 4 + 1) * 128],
                    gates[:],
                    ident_sb[:],
                )

            # ---- phase 1d: expand gates to (e,r) rows, gh^T = gates_er * h^T
            for tb in range(2 if do_p1 else 0):
                sl = slice(tb * 512, (tb + 1) * 512)
                nc.vector.tensor_copy(gT_sb[:, sl], ptr_pack[tb][:])
            for tb in range(2 if do_p1 else 0):
                sl = slice(tb * 512, (tb + 1) * 512)
                pge = psum.tile([128, 512], f32, tag="bank", name=f"pge{tb}")
                nc.tensor.matmul(
                    pge[:], sel_sb[:], gT_sb[:, sl], start=True, stop=True
                )
                nc.vector.tensor_tensor(
                    ghT_sb[:, sl], pge[:], hT_sb[:, sl], op=OP.mult
                )

            if "wdma" in parts:
                wsink = cpool.tile([128, 128], bf16, name="wsink")
                for ot in range(NO2):
                    w_t = wpool.tile([128, NK, 128], bf16, name="w_t")
                    eng = nc.sync if ot % 2 == 0 else nc.scalar
                    eng.dma_start(w_t[:], wt_d[ot])
                    nc.vector.tensor_copy(wsink[:], w_t[:, 0, :])
            if "odma" in parts:
                osrc = opool.tile([128, TPC], f32, name="osrc")
                nc.any.memset(osrc[:], 1.0)
                for ot in range(NO2):
                    oeng = (nc.gpsimd, nc.sync, nc.scalar)[ot % 3]
                    oeng.dma_start(out_d[ot], osrc[:])

            # ---- phase 2: out^T[ot] = W_ot^T-block @ x^T + B_cat_ot @ gh^T
            split = "p2split" in parts
            for ot in range(NO2 if do_p2 else 0):
                w_t = wpool.tile([128, NK, 128], bf16, name="w_t")
                eng = nc.sync if ot % 2 == 0 else nc.scalar
                eng.dma_start(w_t[:], wt_d[ot])
                if do_rt:
                    # 64x128 row-tiled: halves of the contraction run as
                    # concurrent K=64 matmuls on tiles T0 (partitions 0-63)
                    # and T8 (64-127), accumulating in separate PSUM banks;
                    # the evacuation add fuses the two partials.
                    po = [
                        psum.tile([128, 512], f32, tag="bank", name=f"po{ot}_{i}")
                        for i in range(4)
                    ]  # 0: tb0 halfA, 1: tb1 halfA, 2: tb0 halfB, 3: tb1 halfB
                    bsl = bcat_sb[:, ot * 128 : (ot + 1) * 128]
                    for kc in range(NK):
                        first = kc == 0
                        for h2 in range(2):
                            psl = slice(h2 * 64, (h2 + 1) * 64)
                            wsl = w_t[psl, kc, :]
                            for tb in range(2):
                                sl = slice(tb * 512, (tb + 1) * 512)
                                nc.tensor.matmul(
                                    po[2 * h2 + tb][:], wsl, xh_sb[psl, kc, sl],
                                    start=first, stop=False,
                                )
                    for h2 in range(2):
                        psl = slice(h2 * 64, (h2 + 1) * 64)
                        for tb in range(2):
                            sl = slice(tb * 512, (tb + 1) * 512)
                            nc.tensor.matmul(
                                po[2 * h2 + tb][:], bsl[psl, :], ghT_sb[psl, sl],
                                start=False, stop=True,
                            )
                    ob = opool.tile([128, TPC], f32, name="ob")
                    for tb in range(2):
                        sl = slice(tb * 512, (tb + 1) * 512)
                        tmp = gpool.tile([128, 512], f32, name="potmp")
                        nc.scalar.activation(tmp[:], po[2 + tb][:], ACT.Copy)
                        nc.vector.tensor_tensor(
                            ob[:, sl], po[tb][:], tmp[:], op=OP.add
                        )
                    oeng = (nc.gpsimd, nc.sync, nc.scalar)[ot % 3]
                    oeng.dma_start(out_d[ot], ob[:])
                    continue
                po = [
                    psum.tile([128, 512], f32, tag="bank", name=f"po{ot}_{i}")
                    for i in range(2)
                ]
                bsl = bcat_sb[:, ot * 128 : (ot + 1) * 128]
                if split:
                    for tb in range(2):
                        sl = slice(tb * 512, (tb + 1) * 512)
                        for kc in range(NK):
                            nc.tensor.matmul(
                                po[tb][:], w_t[:, kc, :], xh_sb[:, kc, sl],
                                start=(kc == 0), stop=False,
                            )
                        nc.tensor.matmul(
                            po[tb][:], bsl, ghT_sb[:, sl], start=False, stop=True
                        )
                else:
                    for kc in range(NK):
                        wsl = w_t[:, kc, :]
                        first = kc == 0
                        nc.tensor.matmul(
                            po[0][:], wsl, xh_sb[:, kc, 0:512], start=first, stop=False
                        )
                        nc.tensor.matmul(
                            po[1][:], wsl, xh_sb[:, kc, 512:1024],
                            start=first, stop=False,
                        )
                    nc.tensor.matmul(
                        po[0][:], bsl, ghT_sb[:, 0:512], start=False, stop=True
                    )
                    nc.tensor.matmul(
                        po[1][:], bsl, ghT_sb[:, 512:1024], start=False, stop=True
                    )
                ob = opool.tile([128, TPC], f32, name="ob")
                nc.vector.tensor_copy(ob[:, 0:512], po[0][:])
                nc.vector.tensor_copy(ob[:, 512:1024], po[1][:])
                oeng = (nc.gpsimd, nc.sync, nc.scalar)[ot % 3]
                oeng.dma_start(out_d[ot], ob[:])

          if loop_n is None:
              body()
          else:
              with tc.For_i(0, loop_n, 1) as iv:
                  body(iv)

    nc.compile()
    return nc


def _split_bf16(a):
    import ml_dtypes

    hi = a.astype(ml_dtypes.bfloat16)
    lo = (a - hi.astype(np.float32)).astype(ml_dtypes.bfloat16)
    return hi, lo


def _host_prep(x, language_ids, W, Wr, A, B):
    import ml_dtypes

    bf = ml_dtypes.bfloat16
    x = np.asarray(x, dtype=np.float32)
    W = np.asarray(W, dtype=np.float32)
    Wr = np.asarray(Wr, dtype=np.float32)
    A = np.asarray(A, dtype=np.float32)
    B = np.asarray(B, dtype=np.float32)
    lang = np.asarray(language_ids).astype(np.int64)

    xf = np.ascontiguousarray(x.reshape(B_SZ * T_SZ, D_SZ))

    # W^T blocks [ot, p, kc, o], bf16 hi only
    wt = np.ascontiguousarray(
        W.T.astype(bf).reshape(NK, 128, NO2, 128).transpose(2, 1, 0, 3)
    )

    # A_cat^T [p, kc, er]
    acat = np.ascontiguousarray(
        A.reshape(ER, D_SZ).T.astype(bf).reshape(NK, 128, ER).transpose(1, 0, 2)
    )

    # Wr^T hi/lo [2, p, kc, e]
    wrT = Wr.T.astype(np.float32)
    wrh, wrl = _split_bf16(wrT)
    wrt = np.ascontiguousarray(
        np.stack(
            [
                wrh.reshape(NK, 128, E_SZ).transpose(1, 0, 2),
                wrl.reshape(NK, 128, E_SZ).transpose(1, 0, 2),
            ],
            axis=0,
        )
    )

    bcat = np.ascontiguousarray(
        (SCALING * B.transpose(0, 2, 1)).reshape(ER, O_SZ).astype(bf)
    )

    sel = np.zeros((E_SZ, ER), dtype=np.float32)
    sel[np.arange(ER) // R_SZ, np.arange(ER)] = 1.0

    in_maps = []
    for c in range(NCORES):
        shard = xf[c * TPC : (c + 1) * TPC]
        xr = np.ascontiguousarray(shard.T).reshape(NK, 128, TPC)
        xhh, xll = _split_bf16(xr)
        xt = np.ascontiguousarray(np.stack([xhh, xll], axis=0))  # [2, kc, p, t]
        b = int(lang[(c * TPC) // T_SZ])
        biasc = np.zeros((E_SZ, 1), dtype=np.float32)
        if b >= 0:
            biasc[b, 0] = LANG_BIAS
        in_maps.append(
            {
                "xt": xt,
                "wt": wt,
                "wrt": wrt,
                "acat": acat,
                "bcat": bcat,
                "selw": sel,
                "biasc": biasc,
            }
        )
    return in_maps


def kernel(x, language_ids, W, Wr, A, B):
    global LAST_RESULT
    from concourse.bass_utils import run_bass_kernel_spmd

    if "nc" not in _CACHE:
        _CACHE["nc"] = _build_bass()
    nc = _CACHE["nc"]

    in_maps = _host_prep(x, language_ids, W, Wr, A, B)
    res = run_bass_kernel_spmd(nc, in_maps, core_ids=list(range(NCORES)))
    LAST_RESULT = res
    outs = [
        r["out"].transpose(2, 0, 1).reshape(TPC, O_SZ) for r in res.results
    ]
    return np.concatenate(outs, axis=0).reshape(B_SZ, T_SZ, O_SZ)
